# revision 16
# baseline (speedup 1.0000x reference)
"""Trainium2 Bass kernel for nn_AveragePoolingClassLoss.

Reference computation (per image):
  pred = softmax(logits[:, :5], axis=1)            # drop background ch 5
  idx  = argmax_c pred                             # per-pixel class
  s_c  = sum of pred[c] over pixels with idx == c  # == sum of per-pixel max prob
  n_c  = count of pixels with idx == c
  agg  = s_c / n_c (0 if n_c == 0)
  loss = BCE(agg, class_gt), mean over (image, class), log clamp -100

Device algorithm (v8, bf16 ingest):
  - host ships bf16 logits (halves HBM traffic; verified rel err ~2e-4)
  - masks come from RAW logits: argmax(softmax) == argmax(logits), so
    g_c = [l_c == max_c l_c] needs no exp
  - per-pixel max prob via the log identity m = exp(L4 - ln S) where
    L4 = max_c l_c and S = sum_c exp(l_c); kills the reciprocal+multiply
  - exp planes feed ONLY the channel sum S: 3 planes on ACT (exact),
    2 planes via a one-op Schraudolph bit-trick on DVE (bf16 out, ~3%
    sawtooth error that averages out in the 52k-pixel class means)
  - engine balance per image: ACT 3 exps + Ln + Exp(m); DVE 2 fast-exps,
    2 max-tree levels, d-sub, 4 masks (+count accums), 4 diag extracts;
    GPSIMD the other 2 max-tree levels; PE channel-sum + masked-sum traces.
  - class 4 stats by subtraction from totals (sum_m, HW)

Sharding: pure data parallel over the batch: 8 cores x 4 images.
Each core emits the partial BCE numerator sum over its 20 (image, class)
pairs; the host sums the 8 partials and scales.
"""

import numpy as np
import ml_dtypes
from contextlib import ExitStack

import concourse.bass as bass
import concourse.bacc as bacc
import concourse.mybir as mybir
import concourse.tile as tile
from concourse import bass_isa, masks
from concourse.bass_utils import run_bass_kernel_spmd

F32 = mybir.dt.float32
BF16 = mybir.dt.bfloat16
I16 = mybir.dt.int16
ALU = mybir.AluOpType
ACTF = mybir.ActivationFunctionType

N_CORES = 8
IMGS_PER_CORE = 4
N_CLASSES = 5
HW = 512 * 512           # pixels per image
P = 128                  # partitions
FD = HW // P             # 2048 free-dim elements per plane
NSTAT = 9                # per image: [S0..S3, sum_m, G0..G3]
LOG_CLAMP = -100.0

# bf16 Schraudolph exp: exp(x) ~ bf16_bits(round(x * 2^7/ln2 + SCH_B))
SCH_C1 = 184.66496580927726
SCH_B = 16248.5


N_ACT_EXPS = 3       # exp planes on ACT; rest via DVE Schraudolph
PE_SUB = True        # d = L4 - lnS on the PE (else DVE tensor_tensor)


def _build_program(repeat: int = 1, loop_iters: int = 0, unroll: int = 1):
    nc = bacc.Bacc(
        "TRN2",
        target_bir_lowering=False,
        debug=False,
        enable_asserts=False,
        num_devices=N_CORES,
    )

    logits = nc.dram_tensor(
        "logits", [IMGS_PER_CORE, N_CLASSES, 512, 512], BF16, kind="ExternalInput"
    )
    gt = nc.dram_tensor("gt", [IMGS_PER_CORE, N_CLASSES], F32, kind="ExternalInput")
    partial = nc.dram_tensor("partial", [1, 1], F32, kind="ExternalOutput")

    with ExitStack() as ctx:
        tc = ctx.enter_context(tile.TileContext(nc))
        _kernel_body(ctx, tc, logits.ap(), gt.ap(), partial.ap(), repeat, loop_iters,
                     unroll)

    nc.compile()
    return nc


def _kernel_body(ctx, tc, logits, gt, partial, repeat=1, loop_iters=0, unroll=1):
    nc = tc.nc

    lpool = ctx.enter_context(tc.tile_pool(name="planes", bufs=7))
    wpool = ctx.enter_context(tc.tile_pool(name="work", bufs=2))
    xpool = ctx.enter_context(tc.tile_pool(name="xient", bufs=2))
    spool = ctx.enter_context(tc.tile_pool(name="stats", bufs=2))
    pspool = ctx.enter_context(tc.tile_pool(name="psum", bufs=1, space="PSUM"))
    tppool = ctx.enter_context(tc.tile_pool(name="tpsum", bufs=2, space="PSUM"))

    # bf16 identity, used as matmul lhsT for plane summation and diag masks;
    # negative identity for the PE-side subtraction d = L4 - lnS
    ident = spool.tile([P, P], BF16, tag="ident")
    masks.make_identity(nc, ident[:])
    identn = spool.tile([P, P], BF16, tag="identn")
    nc.gpsimd.memset(identn[:], 0.0)
    nc.gpsimd.affine_select(
        out=identn[:], in_=identn[:], compare_op=ALU.not_equal, fill=-1.0,
        base=0, pattern=[[-1, P]], channel_multiplier=1,
    )

    pools = (lpool, wpool, xpool, pspool, tppool, ident, identn)
    stats = spool.tile([P, IMGS_PER_CORE * NSTAT], F32, tag="stats")
    if loop_iters:
        # hardware loop for timing: per-iteration device time is measured
        # via the iteration-count slope between two loop programs
        with tc.For_i(0, loop_iters, 1):
            for _ in range(unroll):
                _images_pass(tc, pools, stats, logits)
    else:
        for rep in range(repeat):
            _images_pass(tc, pools, stats, logits)

    # reduce stats across partitions (every partition ends with the totals)
    allred = spool.tile([P, IMGS_PER_CORE * NSTAT], F32, tag="allred")
    nc.gpsimd.partition_all_reduce(
        allred[:], stats[:], channels=P, reduce_op=bass_isa.ReduceOp.add
    )

    _bce_tail(ctx, tc, allred, gt, partial)


def _images_pass(tc, pools, stats, logits):
    nc = tc.nc
    lpool, wpool, xpool, pspool, tppool, ident, identn = pools
    CH = 512                      # psum chunk columns
    TC = 128                      # trace chunk columns

    for i in range(IMGS_PER_CORE):
        sb = i * NSTAT
        # per-plane DMA (contiguous 512KB bf16 per plane in HBM)
        L = []
        for c in range(N_CLASSES):
            La = lpool.tile([P, FD], BF16, tag="L")
            src = logits[i, c].rearrange("(p a) b -> p (a b)", p=P)
            nc.sync.dma_start(out=La[:], in_=src)
            L.append(La)

        # exp planes (feed only the channel sum): 3 exact on ACT,
        # 2 via the Schraudolph bit-trick on DVE (int16 view of bf16 tile)
        E = []
        for c in range(N_CLASSES):
            Ec = lpool.tile([P, FD], BF16, tag="E")
            if c < N_ACT_EXPS:
                nc.scalar.activation(Ec[:], L[c][:], ACTF.Exp)
            else:
                nc.vector.tensor_scalar(
                    out=Ec[:].bitcast(I16), in0=L[c][:],
                    scalar1=SCH_C1, scalar2=SCH_B,
                    op0=ALU.mult, op1=ALU.add,
                )
            E.append(Ec)

        # max tree over RAW logits on DVE (argmax(softmax) == argmax(logits))
        t01 = xpool.tile([P, FD], BF16, tag="t01")
        t23 = xpool.tile([P, FD], BF16, tag="t23")
        t03 = xpool.tile([P, FD], BF16, tag="t03")
        l4m = wpool.tile([P, FD], BF16, tag="l4m")
        nc.vector.tensor_tensor(t01[:], L[0][:], L[1][:], ALU.max)
        nc.vector.tensor_tensor(t23[:], L[2][:], L[3][:], ALU.max)
        nc.vector.tensor_tensor(t03[:], t01[:], t23[:], ALU.max)
        nc.vector.tensor_tensor(l4m[:], t03[:], L[4][:], ALU.max)

        # sum of the 5 exp planes on the tensor engine (PSUM accumulation)
        ps = pspool.tile([P, FD], F32, tag="S")
        for c in range(N_CLASSES):
            for k in range(FD // CH):
                nc.tensor.matmul(
                    out=ps[:, k * CH:(k + 1) * CH],
                    lhsT=ident[:],
                    rhs=E[c][:, k * CH:(k + 1) * CH],
                    start=(c == 0), stop=(c == N_CLASSES - 1),
                )

        # m = exp(L4 - ln S); the subtraction rides the PE (I@L4 - I@lnS into
        # the reused PSUM tile), so ACT reads PSUM directly and DVE is skipped
        lnS = xpool.tile([P, FD], BF16, tag="lnS")
        nc.scalar.activation(lnS[:], ps[:], ACTF.Ln)
        m = wpool.tile([P, FD], BF16, tag="m")
        if PE_SUB:
            for k in range(FD // CH):
                nc.tensor.matmul(
                    out=ps[:, k * CH:(k + 1) * CH], lhsT=identn[:],
                    rhs=lnS[:, k * CH:(k + 1) * CH], start=True, stop=False,
                )
                nc.tensor.matmul(
                    out=ps[:, k * CH:(k + 1) * CH], lhsT=ident[:],
                    rhs=l4m[:, k * CH:(k + 1) * CH], start=False, stop=True,
                )
            nc.scalar.activation(
                m[:], ps[:], ACTF.Exp, accum_out=stats[:, sb + 4: sb + 5]
            )
        else:
            d = xpool.tile([P, FD], BF16, tag="d")
            nc.vector.tensor_tensor(d[:], l4m[:], lnS[:], ALU.subtract)
            nc.scalar.activation(
                m[:], d[:], ACTF.Exp, accum_out=stats[:, sb + 4: sb + 5]
            )

        # per class 0..3: mask from raw logits with count accumulated free
        gs = []
        for c in range(4):
            g = wpool.tile([P, FD], BF16, tag=f"g{c}")
            nc.vector.scalar_tensor_tensor(
                out=g[:], in0=L[c][:], scalar=1.0, in1=l4m[:],
                op0=ALU.mult, op1=ALU.is_equal,
                accum_out=stats[:, sb + 5 + c: sb + 6 + c],
            )
            gs.append(g)

        # masked sums via PE traces: tp[:, c*128:(c+1)*128] += m_chunk.T @ g_chunk
        tp = tppool.tile([P, 4 * TC], F32, tag="tp")
        nchunks = FD // TC
        for c in range(4):
            for k in range(nchunks):
                nc.tensor.matmul(
                    out=tp[:, c * TC:(c + 1) * TC],
                    lhsT=m[:, k * TC:(k + 1) * TC],
                    rhs=gs[c][:, k * TC:(k + 1) * TC],
                    start=(k == 0), stop=(k == nchunks - 1),
                )
        # S_c = trace(tp_c) = sum over the diagonal; fused mult-by-I + reduce
        for c in range(4):
            dg = xpool.tile([P, TC], F32, tag="dg")
            nc.vector.scalar_tensor_tensor(
                out=dg[:], in0=tp[:, c * TC:(c + 1) * TC], scalar=1.0, in1=ident[:],
                op0=ALU.mult, op1=ALU.mult,
                accum_out=stats[:, sb + c: sb + 1 + c],
            )


def _bce_tail(ctx, tc, allred, gt, partial):
    """Tiny per-core tail on partition 0: build per-(image,class) agg then BCE."""
    nc = tc.nc
    tpool = ctx.enter_context(tc.tile_pool(name="tail", bufs=1))
    NI, NC5 = IMGS_PER_CORE, N_CLASSES
    n20 = NI * NC5

    st = allred[0:1, :]                      # [1, 36]
    st3 = st.rearrange("p (i k) -> p i k", k=NSTAT)  # [1, 4, 9]

    # ssum_i = S0+..+S3 ; gsum_i = G0+..+G3
    ssum = tpool.tile([1, NI], F32, tag="ssum")
    gsum = tpool.tile([1, NI], F32, tag="gsum")
    nc.vector.reduce_sum(ssum[:], st3[:, :, 0:4], axis=mybir.AxisListType.X)
    nc.vector.reduce_sum(gsum[:], st3[:, :, 5:9], axis=mybir.AxisListType.X)

    # s vector A [1, 20] and count vector C [1, 20]
    A = tpool.tile([1, n20], F32, tag="A")
    C = tpool.tile([1, n20], F32, tag="C")
    A3 = A.rearrange("p (i c) -> p i c", c=NC5)
    C3 = C.rearrange("p (i c) -> p i c", c=NC5)
    nc.vector.tensor_copy(A3[:, :, 0:4], st3[:, :, 0:4])
    nc.vector.tensor_copy(C3[:, :, 0:4], st3[:, :, 5:9])
    # class 4 by subtraction from totals
    nc.vector.tensor_tensor(A3[:, :, 4], st3[:, :, 4], ssum[:], ALU.subtract)
    nc.vector.tensor_scalar(
        out=C3[:, :, 4], in0=gsum[:], scalar1=-1.0, scalar2=float(HW),
        op0=ALU.mult, op1=ALU.add,
    )

    # agg = A / max(C, 1)
    nc.vector.tensor_scalar_max(C[:], C[:], 1.0)
    rc = tpool.tile([1, n20], F32, tag="rc")
    nc.vector.reciprocal(rc[:], C[:])
    agg = tpool.tile([1, n20], F32, tag="agg")
    nc.vector.tensor_tensor(agg[:], A[:], rc[:], ALU.mult)

    # logp = clamp(ln(agg)); logq = clamp(ln(1 - agg))
    logp = tpool.tile([1, n20], F32, tag="logp")
    q = tpool.tile([1, n20], F32, tag="q")
    logq = tpool.tile([1, n20], F32, tag="logq")
    nc.scalar.activation(logp[:], agg[:], ACTF.Ln)
    nc.vector.tensor_scalar_max(logp[:], logp[:], LOG_CLAMP)
    nc.vector.tensor_scalar(
        out=q[:], in0=agg[:], scalar1=-1.0, scalar2=1.0, op0=ALU.mult, op1=ALU.add
    )
    nc.scalar.activation(logq[:], q[:], ACTF.Ln)
    nc.vector.tensor_scalar_max(logq[:], logq[:], LOG_CLAMP)

    # terms = gt * logp + (1 - gt) * logq ; partial = sum(terms)
    gtt = tpool.tile([1, n20], F32, tag="gtt")
    nc.sync.dma_start(out=gtt[:], in_=gt.rearrange("(o i) c -> o (i c)", o=1))
    t1 = tpool.tile([1, n20], F32, tag="t1")
    nc.vector.tensor_tensor(t1[:], gtt[:], logp[:], ALU.mult)
    gtc = tpool.tile([1, n20], F32, tag="gtc")
    nc.vector.tensor_scalar(
        out=gtc[:], in0=gtt[:], scalar1=-1.0, scalar2=1.0, op0=ALU.mult, op1=ALU.add
    )
    t2 = tpool.tile([1, n20], F32, tag="t2")
    nc.vector.tensor_tensor(t2[:], gtc[:], logq[:], ALU.mult)
    tsum = tpool.tile([1, n20], F32, tag="tsum")
    nc.vector.tensor_tensor(tsum[:], t1[:], t2[:], ALU.add)
    out = tpool.tile([1, 1], F32, tag="out")
    nc.vector.reduce_sum(out[:], tsum[:], axis=mybir.AxisListType.X)
    nc.sync.dma_start(out=partial[:], in_=out[:])


_NC_CACHE = {}


def _get_program(repeat: int = 1):
    if repeat not in _NC_CACHE:
        _NC_CACHE[repeat] = _build_program(repeat)
    return _NC_CACHE[repeat]


def _get_program_loop(iters: int, unroll: int = 3):
    key = ("loop", iters, unroll)
    if key not in _NC_CACHE:
        _NC_CACHE[key] = _build_program(loop_iters=iters, unroll=unroll)
    return _NC_CACHE[key]


def make_in_maps(segmentation_logits: np.ndarray, class_gt: np.ndarray):
    """Shard + host-side prep: drop background channel, cast to bf16."""
    seg = np.ascontiguousarray(segmentation_logits[:, :N_CLASSES], dtype=np.float32)
    seg16 = seg.astype(ml_dtypes.bfloat16)
    gt = np.ascontiguousarray(class_gt, dtype=np.float32)
    in_maps = []
    for core in range(N_CORES):
        lo = core * IMGS_PER_CORE
        hi = lo + IMGS_PER_CORE
        in_maps.append(
            {
                "logits": np.ascontiguousarray(seg16[lo:hi]),
                "gt": np.ascontiguousarray(gt[lo:hi]),
            }
        )
    return in_maps


def kernel(segmentation_logits: np.ndarray, class_gt: np.ndarray) -> np.ndarray:
    B = segmentation_logits.shape[0]
    assert B == N_CORES * IMGS_PER_CORE

    nc = _get_program()
    in_maps = make_in_maps(segmentation_logits, class_gt)
    results = run_bass_kernel_spmd(nc, in_maps, list(range(N_CORES))).results
    total = sum(float(results[c]["partial"][0, 0]) for c in range(N_CORES))
    loss = -total / (B * N_CLASSES)
    return np.float32(loss)
